# revision 16
# baseline (speedup 1.0000x reference)
"""Sparse BertSelfAttention on 8 trn2 NeuronCores.

Sharding: core c -> batch b = c//4, head-group g = c%4 (heads 4g..4g+3).
Each core computes its batch's QT/KT/V projections for its 4 heads and the
sparse attention (local 128-band + global summary columns), producing the
output column block [2048, 256] for its (batch, head-group).

Sparse structure (STRIDE=128, EXPR=8, L=2048, bidirectional):
  row block bk (rows 128bk..128bk+127):
    - rows 1..127 attend cols [128bk, 128(bk+1)]   (self block + 1 col)
    - row 0 attends cols [128(bk-1), 128bk]        (prev block + 1 col)
  global summary cols (allowed for EVERY row):
    A: cols with (c mod 128) in 120..127  (128 cols, strided AP)
    B: cols 128, 256, ..., 1920           (15 cols)

Shifted-q tiling: the local band is computed per key-block bk against the
q-column window [128bk+1, 128bk+129) (bk=0: [0, 129); bk=15: width 127).
Within that window EVERY q has the same allowed-key set: all of key block
bk minus the global columns (k in 120..127, plus k=0 when bk>=1).  The
row-0-attends-prev-block case is exactly the window's last column.  The
mask is therefore a per-key (= per-partition) bias vector folded into the
exp() activation -- no mask matmuls, no mask DMAs, no prev-block tiles.

Layout trick: scores are computed transposed (S^T[k, q], keys on partitions)
so softmax denominators come from a ones-column appended to V, and
P @ V is computed with lhsT = P^T directly (no transposes anywhere).
exp() skips max-subtraction: allowed scores are O(5), masked underflow to 0.

B-scores: ktgB is zero-padded into a [128, 128] lhsT per head-pair (head h's
15 key vectors at columns 32h+i, its 64 feature rows, zeros elsewhere), so
two ordinary matmuls produce the stacked B-score tile with zeros in unused
partitions (no PSUM memset, no tile_position).  B-PV is one matmul per
(qc, j) against a block-diagonal stacked V (zeros kill the garbage rows).

Engine balance: ACT runs only Exp (+ Copy-with-scale for half the output
normalize) so the activation table never reloads; PSUM evacuation and bias
adds run on DVE.  Scores are emitted two q-chunks ahead of PV so the PE
instruction stream does not stall on ACT exp latency.
"""

import numpy as np
import ml_dtypes

import concourse.bass as bass
from concourse import bacc
import concourse.mybir as mybir
import concourse.tile as tile
from concourse.bass_utils import run_bass_kernel_spmd

BF16 = mybir.dt.bfloat16
F32 = mybir.dt.float32
AF = mybir.ActivationFunctionType

L = 2048
HID = 1024
NB = L // 128  # 16 key blocks
NEG = -10000.0

_prog_cache = {}


def _rep_attnB(v):
    out = np.zeros((128, 1), np.float32)
    for h in range(4):
        out[32 * h : 32 * h + 15, 0] = v
    return out


def _glob_cols():
    # A: (16 blocks) x (8 cols 120..127); B: 128,256,...,1920
    a = (np.arange(16)[:, None] * 128 + 120 + np.arange(8)[None, :]).reshape(-1)
    b = np.arange(1, 16) * 128
    return a, b


def _wblk(bk):
    # q-window width for key block bk
    return 129 if bk == 0 else (127 if bk == 15 else 128)


def _qstart(bk):
    return 0 if bk == 0 else 128 * bk + 1


def build_program(loop_n=None, fast_bias=True, phase="full"):
    nc = bacc.Bacc(None)
    ht_d = nc.dram_tensor("ht", [HID, L], BF16, kind="ExternalInput")
    wq_d = nc.dram_tensor("wq", [HID, 256], BF16, kind="ExternalInput")
    wk_d = nc.dram_tensor("wk", [HID, 256], BF16, kind="ExternalInput")
    wv_d = nc.dram_tensor("wv", [HID, 260], BF16, kind="ExternalInput")
    bqk_d = nc.dram_tensor("bqk", [128, 4], F32, kind="ExternalInput")
    aA_d = nc.dram_tensor("attnA", [128, 1], F32, kind="ExternalInput")
    aB_d = nc.dram_tensor("attnB", [128, 1], F32, kind="ExternalInput")
    mb_d = nc.dram_tensor("mbias", [128, 16], F32, kind="ExternalInput")
    out_d = nc.dram_tensor("out", [L, 256], BF16, kind="ExternalOutput")

    with tile.TileContext(nc) as tc:
        with (
            tc.tile_pool(name="consts", bufs=1) as consts,
            tc.tile_pool(name="pp", bufs=4) as pp,
            tc.tile_pool(name="po", bufs=4) as po,
            tc.tile_pool(name="psmall", bufs=4) as psmall,
            tc.tile_pool(name="psA", bufs=6, space="PSUM") as psA,
            tc.tile_pool(name="psC", bufs=2, space="PSUM") as psC,
        ):
            def emit_exec(si):
                # ---- load inputs; per-set (si) buffers so the unrolled loop
                # body double-buffers the DMAs across logical iterations ----
                htl = []
                for c in range(8):
                    t0_ = consts.tile([128, L], BF16, tag=f"ht{c}s{si}",
                                      name=f"ht{c}s{si}")
                    nc.sync.dma_start(out=t0_, in_=ht_d[128 * c : 128 * c + 128, :])
                    htl.append(t0_)
                wq = consts.tile([128, 8, 256], BF16)
                nc.sync.dma_start(out=wq, in_=wq_d.rearrange("(c p) n -> p c n", p=128))
                wk = consts.tile([128, 8, 256], BF16)
                nc.sync.dma_start(out=wk, in_=wk_d.rearrange("(c p) n -> p c n", p=128))
                wv = consts.tile([128, 8, 260], BF16)
                nc.sync.dma_start(out=wv, in_=wv_d.rearrange("(c p) n -> p c n", p=128))
                bqk = consts.tile([128, 4], F32)
                nc.sync.dma_start(out=bqk, in_=bqk_d[:, :])
                aA = consts.tile([128, 1], F32)
                nc.sync.dma_start(out=aA, in_=aA_d[:, :])
                aB = consts.tile([128, 1], F32)
                nc.sync.dma_start(out=aB, in_=aB_d[:, :])
                mb = consts.tile([128, 16], F32)
                nc.sync.dma_start(out=mb, in_=mb_d[:, :])

                # compact copies of ht's global summary columns (matmul operands
                # must have a single free dim, so gather via DVE first)
                ghtA_sb = consts.tile([128, 8, 128], BF16)
                ghtB_sb = consts.tile([128, 8, 15], BF16)
                for c in range(8):
                    src = htl[c].rearrange("p (a b) -> p a b", b=128)
                    nc.vector.tensor_copy(
                        ghtA_sb[:, c, :].rearrange("p (a b) -> p a b", b=8),
                        src[:, :, 120:128],
                    )
                    nc.vector.tensor_copy(
                        ghtB_sb[:, c, :], src[:, 1:16, 0],
                    )

                def ghtA(c):
                    return ghtA_sb[:, c, :]

                def ghtB(c):
                    return ghtB_sb[:, c, :]

                # ---- projections (emitted interleaved with attention below) --
                qtl = [consts.tile([128, L], BF16, tag=f"qt{t}", name=f"qt{t}")
                       for t in range(2)]
                ktl = [consts.tile([128, L], BF16, tag=f"kt{t}", name=f"kt{t}")
                       for t in range(2)]
                vl = [consts.tile([128, 260], BF16, tag=f"v{blk}", name=f"v{blk}")
                      for blk in range(NB)]
                vgA = consts.tile([128, 260], BF16)
                vgB = consts.tile([128, 260], BF16)
                vgBt = consts.tile([128, 260], BF16, tag="vgBt")
                ktgA = consts.tile([128, 2, 128], BF16)
                ktgBp = consts.tile([128, 2, 128], BF16)

                def emit_qk_chunk(n):
                    # QT and KT over q columns [512n, 512n+512)
                    for dstl, w, bcol in ((qtl, wq, 0), (ktl, wk, 2)):
                        for t in range(2):
                            ps = psA.tile([128, 512], F32, tag="ps")
                            for c in range(8):
                                nc.tensor.matmul(
                                    ps,
                                    lhsT=w[:, c, 128 * t : 128 * t + 128],
                                    rhs=htl[c][:, 512 * n : 512 * n + 512],
                                    start=(c == 0),
                                    stop=(c == 7),
                                )
                            nc.vector.tensor_scalar_add(
                                dstl[t][:, 512 * n : 512 * n + 512],
                                ps,
                                bqk[:, bcol + t : bcol + t + 1],
                            )

                def emit_v_blocks(b0, b1):
                    for blk in range(b0, b1):
                        ps = psA.tile([128, 512], F32, tag="ps")
                        for c in range(8):
                            nc.tensor.matmul(
                                ps[:, 0:260],
                                lhsT=htl[c][:, 128 * blk : 128 * blk + 128],
                                rhs=wv[:, c, :],
                                start=(c == 0),
                                stop=(c == 7),
                            )
                        nc.vector.tensor_copy(vl[blk], ps[:, 0:260])
                        nc.vector.memset(
                            vl[blk].rearrange("p (h d) -> p h d", d=65)[:, :, 64:65],
                            1.0,
                        )

                def emit_globals():
                    # vgA full; vgB block-diagonal stacked
                    ps = psA.tile([128, 512], F32, tag="ps")
                    for c in range(8):
                        nc.tensor.matmul(
                            ps[:, 0:260], lhsT=ghtA(c), rhs=wv[:, c, :],
                            start=(c == 0), stop=(c == 7),
                        )
                    nc.vector.tensor_copy(vgA, ps[:, 0:260])
                    nc.vector.memset(
                        vgA.rearrange("p (h d) -> p h d", d=65)[:, :, 64:65], 1.0
                    )

                    nc.vector.memset(vgB, 0.0)
                    ps = psA.tile([128, 512], F32, tag="ps")
                    for c in range(8):
                        nc.tensor.matmul(
                            ps[0:15, 0:260], lhsT=ghtB(c), rhs=wv[:, c, :],
                            start=(c == 0), stop=(c == 7),
                        )
                    nc.vector.tensor_copy(vgBt[0:15, :], ps[0:15, 0:260])
                    for h in range(4):
                        # cross-partition placement: DMA, not DVE
                        nc.sync.dma_start(
                            out=vgB[32 * h : 32 * h + 15, 65 * h : 65 * h + 64],
                            in_=vgBt[0:15, 65 * h : 65 * h + 64],
                        )
                        nc.vector.memset(
                            vgB[32 * h : 32 * h + 15, 65 * h + 64 : 65 * h + 65],
                            1.0,
                        )

                    # ktgA per pair; ktgB zero-padded per-head stripes
                    nc.vector.memset(ktgBp, 0.0)
                    for t in range(2):
                        ps = psA.tile([128, 512], F32, tag="ps")
                        for c in range(8):
                            nc.tensor.matmul(
                                ps[:, 0:128],
                                lhsT=wk[:, c, 128 * t : 128 * t + 128],
                                rhs=ghtA(c),
                                start=(c == 0),
                                stop=(c == 7),
                            )
                        nc.vector.tensor_scalar_add(
                            ktgA[:, t, :], ps[:, 0:128], bqk[:, 2 + t : 3 + t],
                        )
                        ps = psA.tile([128, 512], F32, tag="ps")
                        for c in range(8):
                            nc.tensor.matmul(
                                ps[:, 0:15],
                                lhsT=wk[:, c, 128 * t : 128 * t + 128],
                                rhs=ghtB(c),
                                start=(c == 0),
                                stop=(c == 7),
                            )
                        for hh in range(2):
                            h = 2 * t + hh
                            nc.vector.tensor_scalar_add(
                                ktgBp[64 * hh : 64 * hh + 64, t, 32 * h : 32 * h + 15],
                                ps[64 * hh : 64 * hh + 64, 0:15],
                                bqk[64 * hh : 64 * hh + 64, 2 + t : 3 + t],
                            )

                # ---- attention, scores emitted 2 q-chunks ahead of PV ----
                # Everything is q-shifted by +1: prob tile col i of chunk qc
                # covers q = 512qc+1+i, and PSUM context block bk covers q
                # rows 128bk+1..128bk+128 (127 rows for bk=15).  The window's
                # last column (block-boundary row attending its prev block)
                # then merges into the main PV matmul.  q=0 has a tiny
                # dedicated path (its self scores are window 0's column 0).
                blk_probs = {}
                gen = {}  # per-qc prob tiles kept alive for the lagged PV

                def emit_scores(qc):
                    qs = 512 * qc + 1
                    qw = 512 if qc < 3 else 511
                    pgB = psA.tile([128, 512], F32, tag="ps")
                    for t in range(2):
                        nc.tensor.matmul(
                            pgB[:, 0:qw],
                            lhsT=ktgBp[:, t, :],
                            rhs=qtl[t][:, qs : qs + qw],
                            start=(t == 0),
                            stop=(t == 1),
                        )
                    pB = pp.tile([128, 512], BF16, tag="pB")
                    nc.scalar.activation(pB[:, 0:qw], pgB[:, 0:qw], AF.Exp, bias=aB)
                    pAs = []
                    for h in range(4):
                        t, hh = h // 2, h % 2
                        p0 = 64 * hh
                        pgA = psA.tile([128, 512], F32, tag="ps")
                        nc.tensor.matmul(
                            pgA[:, 0:qw],
                            lhsT=ktgA[p0 : p0 + 64, t, :],
                            rhs=qtl[t][p0 : p0 + 64, qs : qs + qw],
                            start=True, stop=True,
                        )
                        pA = pp.tile([128, 512], BF16, tag=f"pA{h}")
                        nc.scalar.activation(pA[:, 0:qw], pgA[:, 0:qw],
                                             AF.Exp, bias=aA)
                        pAs.append(pA)
                    # self windows: window bk covers q cols [qstart, qstart+w)
                    # (bk=0: q 0..128 incl. the q=0 column; bk>=1: 128bk+1..)
                    # packed two windows per PSUM tile: X, Y
                    for h in range(4):
                        t, hh = h // 2, h % 2
                        p0 = 64 * hh
                        for half, pair in ((0, (4 * qc, 4 * qc + 1)),
                                           (1, (4 * qc + 2, 4 * qc + 3))):
                            ws = [_wblk(bk) for bk in pair]
                            pss = psA.tile([128, 512], F32, tag="ps")
                            off = 0
                            for bi, bk in enumerate(pair):
                                nc.tensor.matmul(
                                    pss[:, off : off + ws[bi]],
                                    lhsT=ktl[t][p0 : p0 + 64,
                                                128 * bk : 128 * bk + 128],
                                    rhs=qtl[t][p0 : p0 + 64,
                                               _qstart(bk) : _qstart(bk) + ws[bi]],
                                    start=True, stop=True,
                                )
                                off += ws[bi]
                            tot = ws[0] + ws[1]
                            pS = pp.tile([128, 257], BF16, tag=f"pS{half}{h}")
                            if fast_bias:
                                if pair[0] == 0:
                                    # block 0 bias differs (k=0 allowed)
                                    nc.scalar.activation(
                                        pS[:, 0 : ws[0]], pss[:, 0 : ws[0]],
                                        AF.Exp, bias=mb[:, 0:1],
                                    )
                                    nc.scalar.activation(
                                        pS[:, ws[0] : tot], pss[:, ws[0] : tot],
                                        AF.Exp, bias=mb[:, 1:2],
                                    )
                                else:
                                    nc.scalar.activation(
                                        pS[:, 0:tot], pss[:, 0:tot],
                                        AF.Exp, bias=mb[:, 1:2],
                                    )
                            else:
                                off = 0
                                for bi, bk in enumerate(pair):
                                    nc.scalar.activation(
                                        pS[:, off : off + ws[bi]],
                                        pss[:, off : off + ws[bi]],
                                        AF.Exp, bias=mb[:, bk : bk + 1],
                                    )
                                    off += ws[bi]
                            blk_probs[(pair[0], h)] = (pS, 0, ws[0])
                            blk_probs[(pair[1], h)] = (pS, ws[0], ws[1])
                    if qc == 0:
                        # q=0 global scores (A stacked with B in one PSUM tile)
                        pq0 = psA.tile([128, 512], F32, tag="ps")
                        for h in range(4):
                            t, hh = h // 2, h % 2
                            p0 = 64 * hh
                            nc.tensor.matmul(
                                pq0[:, h : h + 1],
                                lhsT=ktgA[p0 : p0 + 64, t, :],
                                rhs=qtl[t][p0 : p0 + 64, 0:1],
                                start=True, stop=True,
                            )
                        for t in range(2):
                            nc.tensor.matmul(
                                pq0[:, 4:5],
                                lhsT=ktgBp[:, t, :],
                                rhs=qtl[t][:, 0:1],
                                start=(t == 0), stop=(t == 1),
                            )
                        pq0e = pp.tile([128, 8], BF16, tag="pq0e")
                        nc.scalar.activation(pq0e[:, 0:4], pq0[:, 0:4],
                                             AF.Exp, bias=aA)
                        nc.scalar.activation(pq0e[:, 4:5], pq0[:, 4:5],
                                             AF.Exp, bias=aB)
                        gen["q0"] = pq0e
                    gen[qc] = (pAs, pB)

                def emit_pv(qc):
                    pAs, pB = gen[qc]
                    if qc == 0:
                        # q = 0 context: A + self(window 0 col 0) + B
                        pq0e = gen["q0"]
                        cx0 = psC.tile([128, 260], F32, tag="cx")
                        # single start=True per PSUM bank (start marks the
                        # whole 2KB bank pending-zero): full-width B first
                        nc.tensor.matmul(
                            cx0[0:1, :],
                            lhsT=pq0e[:, 4:5],
                            rhs=vgB,
                            start=True, stop=False,
                        )
                        for h in range(4):
                            nc.tensor.matmul(
                                cx0[0:1, 65 * h : 65 * h + 65],
                                lhsT=pq0e[:, h : h + 1],
                                rhs=vgA[:, 65 * h : 65 * h + 65],
                                start=False, stop=False,
                            )
                            pS, off, w = blk_probs[(0, h)]
                            nc.tensor.matmul(
                                cx0[0:1, 65 * h : 65 * h + 65],
                                lhsT=pS[:, off : off + 1],
                                rhs=vl[0][:, 65 * h : 65 * h + 65],
                                start=False, stop=(h == 3),
                            )
                        cxv0 = cx0.rearrange("p (h d) -> p h d", d=65)
                        rcp0 = psmall.tile([128, 4], F32, tag="rcp")
                        nc.vector.reciprocal(rcp0[0:1, :], cxv0[0:1, :, 64])
                        outj0 = po.tile([128, 256], BF16, tag="o")
                        for h in range(4):
                            nc.vector.tensor_scalar_mul(
                                outj0[0:1, 64 * h : 64 * h + 64],
                                cxv0[0:1, h, 0:64],
                                rcp0[0:1, h : h + 1],
                            )
                        nc.sync.dma_start(out=out_d[0:1, :], in_=outj0[0:1, :])
                    for j in range(4):
                        bk = 4 * qc + j
                        rows = 128 if bk < 15 else 127
                        cxt = psC.tile([128, 260], F32, tag="cx")
                        # single start=True per PSUM bank (start marks the
                        # whole 2KB bank pending-zero): full-width B first
                        nc.tensor.matmul(
                            cxt[0:rows, :],
                            lhsT=pB[:, 128 * j : 128 * j + rows],
                            rhs=vgB,
                            start=True, stop=False,
                        )
                        for h in range(4):
                            nc.tensor.matmul(
                                cxt[0:rows, 65 * h : 65 * h + 65],
                                lhsT=pAs[h][:, 128 * j : 128 * j + rows],
                                rhs=vgA[:, 65 * h : 65 * h + 65],
                                start=False, stop=False,
                            )
                        for h in range(4):
                            pS, off, w = blk_probs[(bk, h)]
                            # window cols for q rows 128bk+1..: skip the q=0
                            # column of window 0
                            o0 = off + (1 if bk == 0 else 0)
                            nc.tensor.matmul(
                                cxt[0:rows, 65 * h : 65 * h + 65],
                                lhsT=pS[:, o0 : o0 + rows],
                                rhs=vl[bk][:, 65 * h : 65 * h + 65],
                                start=False, stop=(h == 3),
                            )
                        cxv = cxt.rearrange("p (h d) -> p h d", d=65)
                        rcp = psmall.tile([128, 4], F32, tag="rcp")
                        nc.vector.reciprocal(rcp[0:rows, :], cxv[0:rows, :, 64])
                        outj = po.tile([128, 256], BF16, tag="o")
                        for h in range(4):
                            if h < 2:
                                nc.vector.tensor_scalar_mul(
                                    outj[0:rows, 64 * h : 64 * h + 64],
                                    cxv[0:rows, h, 0:64],
                                    rcp[0:rows, h : h + 1],
                                )
                            else:
                                nc.scalar.activation(
                                    outj[0:rows, 64 * h : 64 * h + 64],
                                    cxv[0:rows, h, 0:64],
                                    AF.Copy,
                                    scale=rcp[0:rows, h : h + 1],
                                )
                        nc.sync.dma_start(
                            out=out_d[128 * bk + 1 : 128 * bk + 1 + rows, :],
                            in_=outj[0:rows, :],
                        )

                # interleaved schedule: projection chunks feed the PE stream
                # while ACT runs the previous chunk's exps; PV trails by one.
                # scores(qc) needs q column 512qc+512 (the +1 shift), hence
                # runs after projection chunk qc+1.
                if phase == "proj":
                    # diagnostic: projections only; DMA qtl out so nothing is
                    # dead-code eliminated
                    for n in range(4):
                        emit_qk_chunk(n)
                    emit_v_blocks(0, 16)
                    emit_globals()
                    for t in range(2):
                        nc.sync.dma_start(
                            out=out_d[512 * t : 512 * t + 128, :],
                            in_=qtl[t][:, 0:256],
                        )
                        nc.sync.dma_start(
                            out=out_d[512 * t + 128 : 512 * t + 256, :],
                            in_=ktl[t][:, 0:256],
                        )
                elif phase == "noscore":
                    for n in range(4):
                        emit_qk_chunk(n)
                    emit_v_blocks(0, 16)
                    emit_globals()
                    for qc in range(4):
                        emit_scores(qc)
                    for h in range(4):
                        nc.sync.dma_start(
                            out=out_d[128 * h : 128 * h + 128, :],
                            in_=gen[3][0][h][:, 0:256],
                        )
                else:
                    emit_qk_chunk(0)
                    emit_v_blocks(0, 4)
                    emit_globals()
                    emit_qk_chunk(1)
                    emit_scores(0)
                    emit_v_blocks(4, 8)
                    emit_qk_chunk(2)
                    emit_scores(1)
                    emit_pv(0)
                    emit_v_blocks(8, 12)
                    emit_qk_chunk(3)
                    emit_scores(2)
                    emit_pv(1)
                    emit_v_blocks(12, 16)
                    emit_scores(3)
                    emit_pv(2)
                    emit_pv(3)

            if loop_n == -1:
                # straight-line unroll for TimelineSim (no hw loop support)
                for i in range(6):
                    emit_exec(i % 2)
            elif loop_n:
                assert loop_n % 2 == 0, "loop_n must be even (unroll-2 body)"
                with tc.For_i(0, loop_n // 2, 1):
                    emit_exec(0)
                    emit_exec(1)
            else:
                emit_exec(0)
    nc.finalize()
    return nc


def _prepare_inputs(hidden_states, attention_mask, Wq, bq, Wk, bk, Wv, bv, sparse_mask):
    bf = ml_dtypes.bfloat16
    hs = np.asarray(hidden_states, np.float32)
    am = np.asarray(attention_mask, np.float32).reshape(2, L)
    Wq = np.asarray(Wq, np.float32)
    Wk = np.asarray(Wk, np.float32)
    Wv = np.asarray(Wv, np.float32)
    bq = np.asarray(bq, np.float32)
    bk = np.asarray(bk, np.float32)
    bv = np.asarray(bv, np.float32)
    gA, gB = _glob_cols()

    in_maps = []
    per_batch = {}
    fast = True
    for b in range(2):
        ht = np.ascontiguousarray(hs[b].T).astype(bf)  # [1024, 2048]
        # per-block self-window bias: -1e4 at excluded keys (A cols 120..127
        # always; k=0 when bk>=1 since col 128bk is a B global), plus the
        # additive attention mask at key 128bk+k.
        mbias = np.zeros((128, 16), np.float32)
        for blk in range(NB):
            mbias[:, blk] = am[b][128 * blk : 128 * blk + 128]
            mbias[120:128, blk] += NEG
            if blk >= 1:
                mbias[0, blk] += NEG
        if not np.all(mbias[:, 1:] == mbias[:, 1:2]):
            fast = False
        per_batch[b] = (
            ht,
            mbias,
            am[b][gA].reshape(128, 1).copy(),
            _rep_attnB(am[b][gB]),
        )

    for core in range(8):
        b, g = core // 4, core % 4
        ht, mbias, aAv, aBv = per_batch[b]
        cols = slice(256 * g, 256 * g + 256)
        wq = (Wq[:, cols] * 0.125).astype(bf)
        wk_ = Wk[:, cols].astype(bf)
        wv_ = np.zeros((HID, 260), np.float32)
        for j in range(4):
            wv_[:, 65 * j : 65 * j + 64] = Wv[:, cols.start + 64 * j : cols.start + 64 * j + 64]
        bqk_ = np.stack(
            [
                bq[cols][:128] * 0.125,
                bq[cols][128:] * 0.125,
                bk[cols][:128],
                bk[cols][128:],
            ],
            axis=1,
        ).astype(np.float32)
        in_maps.append(
            dict(
                ht=ht,
                wq=wq,
                wk=wk_,
                wv=wv_.astype(bf),
                bqk=np.ascontiguousarray(bqk_),
                attnA=aAv,
                attnB=aBv,
                mbias=np.ascontiguousarray(mbias),
            )
        )
    # NOTE: bv is folded nowhere: it is zeros by construction in this problem.
    assert np.all(bv == 0.0), "kernel assumes zero V bias"
    return in_maps, fast


def kernel(hidden_states, attention_mask, Wq, bq, Wk, bk, Wv, bv, sparse_mask,
           trace=False):
    in_maps, fast = _prepare_inputs(
        hidden_states, attention_mask, Wq, bq, Wk, bk, Wv, bv, sparse_mask
    )
    key = ("nc", fast)
    if key not in _prog_cache:
        _prog_cache[key] = build_program(fast_bias=fast)
    nc = _prog_cache[key]
    res = run_bass_kernel_spmd(nc, in_maps, list(range(8)), trace=trace)
    out = np.empty((2, L, HID), np.float32)
    for core in range(8):
        b, g = core // 4, core % 4
        out[b][:, 256 * g : 256 * g + 256] = np.asarray(
            res.results[core]["out"], np.float32
        )
    if trace:
        _prog_cache["last_results"] = res
    return out


# revision 28
# speedup vs baseline: 1.1954x; 1.1954x over previous
"""Sparse BertSelfAttention on 8 trn2 NeuronCores.

Sharding: core c -> batch b = c//4, head-group g = c%4 (heads 4g..4g+3).
Each core computes its batch's QT/KT/V projections for its 4 heads and the
sparse attention (local 128-band + global summary columns), producing the
output column block [2048, 256] for its (batch, head-group).

Sparse structure (STRIDE=128, EXPR=8, L=2048, bidirectional):
  row block bk (rows 128bk..128bk+127):
    - rows 1..127 attend cols [128bk, 128(bk+1)]   (self block + 1 col)
    - row 0 attends cols [128(bk-1), 128bk]        (prev block + 1 col)
  global summary cols (allowed for EVERY row):
    A: cols with (c mod 128) in 120..127  (128 cols, strided AP)
    B: cols 128, 256, ..., 1920           (15 cols)

Shifted-q tiling: the local band is computed per key-block bk against the
q-column window [128bk+1, 128bk+129) (bk=0: [0, 129); bk=15: width 127).
Within that window EVERY q has the same allowed-key set: all of key block
bk minus the global columns (k in 120..127, plus k=0 when bk>=1).  The
row-0-attends-prev-block case is exactly the window's last column.  The
mask is therefore a per-key (= per-partition) bias vector folded into the
exp() activation -- no mask matmuls, no mask DMAs, no prev-block tiles.

Layout trick: scores are computed transposed (S^T[k, q], keys on partitions)
so softmax denominators come from a ones-column appended to V, and
P @ V is computed with lhsT = P^T directly (no transposes anywhere).
exp() skips max-subtraction: allowed scores are O(5), masked underflow to 0.

B-scores: ktgB is zero-padded into a [128, 128] lhsT per head-pair (head h's
15 key vectors at columns 32h+i, its 64 feature rows, zeros elsewhere), so
two ordinary matmuls produce the stacked B-score tile with zeros in unused
partitions (no PSUM memset, no tile_position).  B-PV is one matmul per
(qc, j) against a block-diagonal stacked V (zeros kill the garbage rows).

Engine balance: ACT runs only Exp (+ Copy-with-scale for half the output
normalize) so the activation table never reloads; PSUM evacuation and bias
adds run on DVE.  Scores are emitted two q-chunks ahead of PV so the PE
instruction stream does not stall on ACT exp latency.
"""

import numpy as np
import ml_dtypes

import concourse.bass as bass
from concourse import bacc
import concourse.mybir as mybir
import concourse.tile as tile
from concourse.bass_utils import run_bass_kernel_spmd

BF16 = mybir.dt.bfloat16
F32 = mybir.dt.float32
AF = mybir.ActivationFunctionType

L = 2048
HID = 1024
NB = L // 128  # 16 key blocks
NEG = -10000.0

_prog_cache = {}


def _rep_attnB(v):
    out = np.zeros((128, 1), np.float32)
    for h in range(4):
        out[32 * h : 32 * h + 15, 0] = v
    return out


def _glob_cols():
    # A: (16 blocks) x (8 cols 120..127); B: 128,256,...,1920
    a = (np.arange(16)[:, None] * 128 + 120 + np.arange(8)[None, :]).reshape(-1)
    b = np.arange(1, 16) * 128
    return a, b


def _wblk(bk):
    # q-window width for key block bk
    return 129 if bk == 0 else (127 if bk == 15 else 128)


def _qstart(bk):
    return 0 if bk == 0 else 128 * bk + 1


def build_program(loop_n=None, fast_bias=True, phase="full", pools=(3, 3, 2), pp_bufs=4):
    nc = bacc.Bacc(None)
    ht_d = nc.dram_tensor("ht", [HID, L], BF16, kind="ExternalInput")
    wq_d = nc.dram_tensor("wq", [HID, 256], BF16, kind="ExternalInput")
    wk_d = nc.dram_tensor("wk", [HID, 256], BF16, kind="ExternalInput")
    wv_d = nc.dram_tensor("wv", [HID, 260], BF16, kind="ExternalInput")
    bqk_d = nc.dram_tensor("bqk", [128, 4], F32, kind="ExternalInput")
    aA_d = nc.dram_tensor("attnA", [128, 1], F32, kind="ExternalInput")
    aB_d = nc.dram_tensor("attnB", [128, 1], F32, kind="ExternalInput")
    mb_d = nc.dram_tensor("mbias", [128, 16], F32, kind="ExternalInput")
    out_d = nc.dram_tensor("out", [L, 256], BF16, kind="ExternalOutput")

    with tile.TileContext(nc) as tc:
        with (
            tc.tile_pool(name="consts", bufs=1) as consts,
            tc.tile_pool(name="pp", bufs=pp_bufs) as pp,
            tc.tile_pool(name="po", bufs=4) as po,
            tc.tile_pool(name="psmall", bufs=4) as psmall,
            tc.tile_pool(name="psP", bufs=pools[0], space="PSUM") as psP,
            tc.tile_pool(name="psS", bufs=pools[1], space="PSUM") as psS,
            tc.tile_pool(name="psC", bufs=pools[2], space="PSUM") as psC,
        ):
            def emit_exec(si):
                # ---- load inputs; per-set (si) buffers so the unrolled loop
                # body double-buffers the DMAs across logical iterations ----
                htl = []
                for c in range(8):
                    t0_ = consts.tile([128, L], BF16, tag=f"ht{c}s{si}",
                                      name=f"ht{c}s{si}")
                    nc.sync.dma_start(out=t0_, in_=ht_d[128 * c : 128 * c + 128, :])
                    htl.append(t0_)
                wq = consts.tile([128, 8, 256], BF16)
                nc.sync.dma_start(out=wq, in_=wq_d.rearrange("(c p) n -> p c n", p=128))
                wk = consts.tile([128, 8, 256], BF16)
                nc.sync.dma_start(out=wk, in_=wk_d.rearrange("(c p) n -> p c n", p=128))
                wv = consts.tile([128, 8, 260], BF16)
                nc.sync.dma_start(out=wv, in_=wv_d.rearrange("(c p) n -> p c n", p=128))
                bqk = consts.tile([128, 4], F32)
                nc.sync.dma_start(out=bqk, in_=bqk_d[:, :])
                aA = consts.tile([128, 1], F32)
                nc.sync.dma_start(out=aA, in_=aA_d[:, :])
                aB = consts.tile([128, 1], F32)
                nc.sync.dma_start(out=aB, in_=aB_d[:, :])
                mb = consts.tile([128, 16], F32)
                nc.sync.dma_start(out=mb, in_=mb_d[:, :])

                # compact copies of ht's global summary columns (matmul operands
                # must have a single free dim, so gather via DVE first)
                ghtA_sb = consts.tile([128, 8, 128], BF16)
                ghtB_sb = consts.tile([128, 8, 15], BF16)
                for c in range(8):
                    src = htl[c].rearrange("p (a b) -> p a b", b=128)
                    nc.vector.tensor_copy(
                        ghtA_sb[:, c, :].rearrange("p (a b) -> p a b", b=8),
                        src[:, :, 120:128],
                    )
                    nc.vector.tensor_copy(
                        ghtB_sb[:, c, :], src[:, 1:16, 0],
                    )

                def ghtA(c):
                    return ghtA_sb[:, c, :]

                def ghtB(c):
                    return ghtB_sb[:, c, :]

                # ---- projections (emitted interleaved with attention below) --
                qtl = [consts.tile([128, L], BF16, tag=f"qt{t}", name=f"qt{t}")
                       for t in range(2)]
                ktl = [consts.tile([128, L], BF16, tag=f"kt{t}", name=f"kt{t}")
                       for t in range(2)]
                vl = [consts.tile([128, 260], BF16, tag=f"v{blk}", name=f"v{blk}")
                      for blk in range(NB)]
                vgA = consts.tile([128, 260], BF16)
                vgB = consts.tile([128, 260], BF16)
                vgBt = consts.tile([128, 260], BF16, tag="vgBt")
                ktgA = consts.tile([128, 2, 128], BF16)
                ktgBp = consts.tile([128, 2, 128], BF16)

                def emit_qk_part(n, which, t):
                    # one projection group: Q (which=0) or K (which=1), pair t,
                    # over q columns [512n, 512n+512)
                    dstl, w, bcol = ((qtl, wq, 0), (ktl, wk, 2))[which]
                    ps = psP.tile([128, 512], F32, tag="ps")
                    for c in range(8):
                        nc.tensor.matmul(
                            ps,
                            lhsT=w[:, c, 128 * t : 128 * t + 128],
                            rhs=htl[c][:, 512 * n : 512 * n + 512],
                            start=(c == 0),
                            stop=(c == 7),
                        )
                    nc.vector.tensor_scalar_add(
                        dstl[t][:, 512 * n : 512 * n + 512],
                        ps,
                        bqk[:, bcol + t : bcol + t + 1],
                    )

                def emit_qk_chunk(n):
                    for which in range(2):
                        for t in range(2):
                            emit_qk_part(n, which, t)

                def emit_v_blocks(b0, b1):
                    for blk in range(b0, b1):
                        ps = psP.tile([128, 512], F32, tag="ps")
                        for c in range(8):
                            nc.tensor.matmul(
                                ps[:, 0:260],
                                lhsT=htl[c][:, 128 * blk : 128 * blk + 128],
                                rhs=wv[:, c, :],
                                start=(c == 0),
                                stop=(c == 7),
                            )
                        nc.vector.tensor_copy(vl[blk], ps[:, 0:260])
                        nc.vector.memset(
                            vl[blk].rearrange("p (h d) -> p h d", d=65)[:, :, 64:65],
                            1.0,
                        )

                def emit_globals():
                    # vgA full; vgB block-diagonal stacked
                    ps = psP.tile([128, 512], F32, tag="ps")
                    for c in range(8):
                        nc.tensor.matmul(
                            ps[:, 0:260], lhsT=ghtA(c), rhs=wv[:, c, :],
                            start=(c == 0), stop=(c == 7),
                        )
                    nc.vector.tensor_copy(vgA, ps[:, 0:260])
                    nc.vector.memset(
                        vgA.rearrange("p (h d) -> p h d", d=65)[:, :, 64:65], 1.0
                    )

                    nc.vector.memset(vgB, 0.0)
                    ps = psP.tile([128, 512], F32, tag="ps")
                    for c in range(8):
                        nc.tensor.matmul(
                            ps[0:15, 0:260], lhsT=ghtB(c), rhs=wv[:, c, :],
                            start=(c == 0), stop=(c == 7),
                        )
                    nc.vector.tensor_copy(vgBt[0:15, :], ps[0:15, 0:260])
                    for h in range(4):
                        # cross-partition placement: DMA, not DVE
                        nc.sync.dma_start(
                            out=vgB[32 * h : 32 * h + 15, 65 * h : 65 * h + 64],
                            in_=vgBt[0:15, 65 * h : 65 * h + 64],
                        )
                        nc.vector.memset(
                            vgB[32 * h : 32 * h + 15, 65 * h + 64 : 65 * h + 65],
                            1.0,
                        )

                    # ktgA per pair; ktgB zero-padded per-head stripes
                    nc.vector.memset(ktgBp, 0.0)
                    for t in range(2):
                        ps = psP.tile([128, 512], F32, tag="ps")
                        for c in range(8):
                            nc.tensor.matmul(
                                ps[:, 0:128],
                                lhsT=wk[:, c, 128 * t : 128 * t + 128],
                                rhs=ghtA(c),
                                start=(c == 0),
                                stop=(c == 7),
                            )
                        nc.vector.tensor_scalar_add(
                            ktgA[:, t, :], ps[:, 0:128], bqk[:, 2 + t : 3 + t],
                        )
                        ps = psP.tile([128, 512], F32, tag="ps")
                        for c in range(8):
                            nc.tensor.matmul(
                                ps[:, 0:15],
                                lhsT=wk[:, c, 128 * t : 128 * t + 128],
                                rhs=ghtB(c),
                                start=(c == 0),
                                stop=(c == 7),
                            )
                        for hh in range(2):
                            h = 2 * t + hh
                            nc.vector.tensor_scalar_add(
                                ktgBp[64 * hh : 64 * hh + 64, t, 32 * h : 32 * h + 15],
                                ps[64 * hh : 64 * hh + 64, 0:15],
                                bqk[64 * hh : 64 * hh + 64, 2 + t : 3 + t],
                            )

                # ---- attention, scores emitted 2 q-chunks ahead of PV ----
                # Everything is q-shifted by +1: prob tile col i of chunk qc
                # covers q = 512qc+1+i, and PSUM context block bk covers q
                # rows 128bk+1..128bk+128 (127 rows for bk=15).  The window's
                # last column (block-boundary row attending its prev block)
                # then merges into the main PV matmul.  q=0 has a tiny
                # dedicated path (its self scores are window 0's column 0).
                blk_probs = {}
                gen = {}  # per-qc prob tiles kept alive for the lagged PV

                def emit_scores_B(qc):
                    qs = 512 * qc + 1
                    qw = 512 if qc < 3 else 511
                    pgB = psS.tile([128, 512], F32, tag="ss")
                    for t in range(2):
                        nc.tensor.matmul(
                            pgB[:, 0:qw],
                            lhsT=ktgBp[:, t, :],
                            rhs=qtl[t][:, qs : qs + qw],
                            start=(t == 0),
                            stop=(t == 1),
                        )
                    pB = pp.tile([128, 512], BF16, tag="pB")
                    nc.scalar.activation(pB[:, 0:qw], pgB[:, 0:qw], AF.Exp, bias=aB)
                    gen.setdefault(qc, {})["pB"] = pB

                def emit_scores_A(qc, h):
                    qs = 512 * qc + 1
                    qw = 512 if qc < 3 else 511
                    t, hh = h // 2, h % 2
                    p0 = 64 * hh
                    pgA = psS.tile([128, 512], F32, tag="ss")
                    nc.tensor.matmul(
                        pgA[:, 0:qw],
                        lhsT=ktgA[p0 : p0 + 64, t, :],
                        rhs=qtl[t][p0 : p0 + 64, qs : qs + qw],
                        start=True, stop=True,
                    )
                    pA = pp.tile([128, 512], BF16, tag=f"pA{h}")
                    nc.scalar.activation(pA[:, 0:qw], pgA[:, 0:qw],
                                         AF.Exp, bias=aA)
                    gen.setdefault(qc, {}).setdefault("pAs", {})[h] = pA

                def emit_scores_win(qc, h):
                    # self windows: window bk covers q cols [qstart, qstart+w)
                    # (bk=0: q 0..128 incl. the q=0 column; bk>=1: 128bk+1..).
                    # qc>=1: all four windows fit one PSUM bank (<=512 cols);
                    # qc==0 is 513 cols total, so split into X (2) + Y (2).
                    t, hh = h // 2, h % 2
                    p0 = 64 * hh
                    groups = ([(4 * qc, 4 * qc + 1), (4 * qc + 2, 4 * qc + 3)]
                              if qc == 0 else
                              [tuple(range(4 * qc, 4 * qc + 4))])
                    for gi, grp in enumerate(groups):
                        ws = [_wblk(bk) for bk in grp]
                        tot = sum(ws)
                        pss = psS.tile([128, 512], F32, tag="ss")
                        off = 0
                        for bi, bk in enumerate(grp):
                            nc.tensor.matmul(
                                pss[:, off : off + ws[bi]],
                                lhsT=ktl[t][p0 : p0 + 64,
                                            128 * bk : 128 * bk + 128],
                                rhs=qtl[t][p0 : p0 + 64,
                                           _qstart(bk) : _qstart(bk) + ws[bi]],
                                start=True, stop=True,
                            )
                            off += ws[bi]
                        pS = pp.tile([128, 512], BF16, tag=f"pS{gi}{h}")
                        if fast_bias:
                            if grp[0] == 0:
                                # block 0 bias differs (k=0 allowed)
                                nc.scalar.activation(
                                    pS[:, 0 : ws[0]], pss[:, 0 : ws[0]],
                                    AF.Exp, bias=mb[:, 0:1],
                                )
                                nc.scalar.activation(
                                    pS[:, ws[0] : tot], pss[:, ws[0] : tot],
                                    AF.Exp, bias=mb[:, 1:2],
                                )
                            else:
                                nc.scalar.activation(
                                    pS[:, 0:tot], pss[:, 0:tot],
                                    AF.Exp, bias=mb[:, 1:2],
                                )
                        else:
                            off = 0
                            for bi, bk in enumerate(grp):
                                nc.scalar.activation(
                                    pS[:, off : off + ws[bi]],
                                    pss[:, off : off + ws[bi]],
                                    AF.Exp, bias=mb[:, bk : bk + 1],
                                )
                                off += ws[bi]
                        off = 0
                        for bi, bk in enumerate(grp):
                            blk_probs[(bk, h)] = (pS, off, ws[bi])
                            off += ws[bi]

                def emit_scores_q0():
                    # q=0 global scores (A stacked with B in one PSUM tile)
                    pq0 = psS.tile([128, 512], F32, tag="ss")
                    for h in range(4):
                        t, hh = h // 2, h % 2
                        p0 = 64 * hh
                        nc.tensor.matmul(
                            pq0[:, h : h + 1],
                            lhsT=ktgA[p0 : p0 + 64, t, :],
                            rhs=qtl[t][p0 : p0 + 64, 0:1],
                            start=True, stop=True,
                        )
                    for t in range(2):
                        nc.tensor.matmul(
                            pq0[:, 4:5],
                            lhsT=ktgBp[:, t, :],
                            rhs=qtl[t][:, 0:1],
                            start=(t == 0), stop=(t == 1),
                        )
                    pq0e = pp.tile([128, 8], BF16, tag="pq0e")
                    nc.scalar.activation(pq0e[:, 0:4], pq0[:, 0:4],
                                         AF.Exp, bias=aA)
                    nc.scalar.activation(pq0e[:, 4:5], pq0[:, 4:5],
                                         AF.Exp, bias=aB)
                    gen["q0"] = pq0e

                def emit_scores(qc):
                    emit_scores_B(qc)
                    for h in range(4):
                        emit_scores_A(qc, h)
                    for h in range(4):
                        emit_scores_win(qc, h)
                    if qc == 0:
                        emit_scores_q0()

                def emit_pv_q0():
                    if True:
                        # q = 0 context: A + self(window 0 col 0) + B
                        pq0e = gen["q0"]
                        cx0 = psC.tile([128, 260], F32, tag="cx")
                        # single start=True per PSUM bank (start marks the
                        # whole 2KB bank pending-zero): full-width B first
                        nc.tensor.matmul(
                            cx0[0:1, :],
                            lhsT=pq0e[:, 4:5],
                            rhs=vgB,
                            start=True, stop=False,
                        )
                        for h in range(4):
                            nc.tensor.matmul(
                                cx0[0:1, 65 * h : 65 * h + 65],
                                lhsT=pq0e[:, h : h + 1],
                                rhs=vgA[:, 65 * h : 65 * h + 65],
                                start=False, stop=False,
                            )
                            pS, off, w = blk_probs[(0, h)]
                            nc.tensor.matmul(
                                cx0[0:1, 65 * h : 65 * h + 65],
                                lhsT=pS[:, off : off + 1],
                                rhs=vl[0][:, 65 * h : 65 * h + 65],
                                start=False, stop=(h == 3),
                            )
                        cxv0 = cx0.rearrange("p (h d) -> p h d", d=65)
                        rcp0 = psmall.tile([128, 4], F32, tag="rcp")
                        nc.vector.reciprocal(rcp0[0:1, :], cxv0[0:1, :, 64])
                        outj0 = po.tile([128, 256], BF16, tag="o")
                        for h in range(4):
                            nc.vector.tensor_scalar_mul(
                                outj0[0:1, 64 * h : 64 * h + 64],
                                cxv0[0:1, h, 0:64],
                                rcp0[0:1, h : h + 1],
                            )
                        nc.sync.dma_start(out=out_d[0:1, :], in_=outj0[0:1, :])

                def emit_pv_j(qc, j):
                    pAs, pB = gen[qc]["pAs"], gen[qc]["pB"]
                    if True:
                        bk = 4 * qc + j
                        rows = 128 if bk < 15 else 127
                        cxt = psC.tile([128, 260], F32, tag="cx")
                        # single start=True per PSUM bank (start marks the
                        # whole 2KB bank pending-zero): full-width B first
                        nc.tensor.matmul(
                            cxt[0:rows, :],
                            lhsT=pB[:, 128 * j : 128 * j + rows],
                            rhs=vgB,
                            start=True, stop=False,
                        )
                        for h in range(4):
                            nc.tensor.matmul(
                                cxt[0:rows, 65 * h : 65 * h + 65],
                                lhsT=pAs[h][:, 128 * j : 128 * j + rows],
                                rhs=vgA[:, 65 * h : 65 * h + 65],
                                start=False, stop=False,
                            )
                        for h in range(4):
                            pS, off, w = blk_probs[(bk, h)]
                            # window cols for q rows 128bk+1..: skip the q=0
                            # column of window 0
                            o0 = off + (1 if bk == 0 else 0)
                            nc.tensor.matmul(
                                cxt[0:rows, 65 * h : 65 * h + 65],
                                lhsT=pS[:, o0 : o0 + rows],
                                rhs=vl[bk][:, 65 * h : 65 * h + 65],
                                start=False, stop=(h == 3),
                            )
                        cxv = cxt.rearrange("p (h d) -> p h d", d=65)
                        rcp = psmall.tile([128, 4], F32, tag="rcp")
                        nc.vector.reciprocal(rcp[0:rows, :], cxv[0:rows, :, 64])
                        outj = po.tile([128, 256], BF16, tag="o")
                        for h in range(4):
                            if h < 2:
                                nc.vector.tensor_scalar_mul(
                                    outj[0:rows, 64 * h : 64 * h + 64],
                                    cxv[0:rows, h, 0:64],
                                    rcp[0:rows, h : h + 1],
                                )
                            else:
                                nc.scalar.activation(
                                    outj[0:rows, 64 * h : 64 * h + 64],
                                    cxv[0:rows, h, 0:64],
                                    AF.Copy,
                                    scale=rcp[0:rows, h : h + 1],
                                )
                        nc.sync.dma_start(
                            out=out_d[128 * bk + 1 : 128 * bk + 1 + rows, :],
                            in_=outj[0:rows, :],
                        )

                def emit_pv(qc):
                    if qc == 0:
                        emit_pv_q0()
                    for j in range(4):
                        emit_pv_j(qc, j)

                # interleaved schedule: projection chunks feed the PE stream
                # while ACT runs the previous chunk's exps; PV trails by one.
                # scores(qc) needs q column 512qc+512 (the +1 shift), hence
                # runs after projection chunk qc+1.
                if phase == "proj":
                    # diagnostic: projections only; DMA qtl out so nothing is
                    # dead-code eliminated
                    for n in range(4):
                        emit_qk_chunk(n)
                    emit_v_blocks(0, 16)
                    emit_globals()
                    for t in range(2):
                        nc.sync.dma_start(
                            out=out_d[512 * t : 512 * t + 128, :],
                            in_=qtl[t][:, 0:256],
                        )
                        nc.sync.dma_start(
                            out=out_d[512 * t + 128 : 512 * t + 256, :],
                            in_=ktl[t][:, 0:256],
                        )
                elif phase == "noscore":
                    for n in range(4):
                        emit_qk_chunk(n)
                    emit_v_blocks(0, 16)
                    emit_globals()
                    for qc in range(4):
                        emit_scores(qc)
                    for h in range(4):
                        nc.sync.dma_start(
                            out=out_d[128 * h : 128 * h + 128, :],
                            in_=gen[3]["pAs"][h][:, 0:256],
                        )
                else:
                    # fine-grained weave: score/PV units are spaced with
                    # projection groups so score PSUM banks (freed by ACT
                    # exps) recycle without stalling the in-order PE stream.
                    emit_qk_chunk(0)
                    emit_v_blocks(0, 2)
                    emit_v_blocks(2, 4)
                    emit_globals()
                    emit_qk_chunk(1)

                    emit_scores_B(0);    emit_qk_part(2, 0, 0)
                    emit_scores_A(0, 0); emit_qk_part(2, 0, 1)
                    emit_scores_A(0, 1); emit_v_blocks(4, 6)
                    emit_scores_A(0, 2); emit_v_blocks(6, 8)
                    emit_scores_A(0, 3); emit_qk_part(2, 1, 0)
                    emit_scores_win(0, 0); emit_qk_part(2, 1, 1)
                    emit_scores_win(0, 1); emit_v_blocks(8, 10)
                    emit_scores_win(0, 2); emit_v_blocks(10, 12)
                    emit_scores_win(0, 3); emit_scores_q0()

                    emit_scores_B(1);    emit_qk_part(3, 0, 0)
                    emit_scores_A(1, 0); emit_qk_part(3, 0, 1)
                    emit_scores_A(1, 1); emit_qk_part(3, 1, 0)
                    emit_scores_A(1, 2); emit_qk_part(3, 1, 1)
                    emit_scores_A(1, 3); emit_v_blocks(12, 14)
                    emit_scores_win(1, 0); emit_v_blocks(14, 16)
                    emit_scores_win(1, 1); emit_pv_q0()
                    emit_scores_win(1, 2); emit_pv_j(0, 0)
                    emit_scores_win(1, 3); emit_pv_j(0, 1)
                    emit_pv_j(0, 2)
                    emit_pv_j(0, 3)

                    emit_scores_B(2)
                    emit_scores_A(2, 0); emit_pv_j(1, 0)
                    emit_scores_A(2, 1); emit_pv_j(1, 1)
                    emit_scores_A(2, 2); emit_pv_j(1, 2)
                    emit_scores_A(2, 3); emit_pv_j(1, 3)
                    emit_scores_win(2, 0)
                    emit_scores_win(2, 1)
                    emit_scores_win(2, 2)
                    emit_scores_win(2, 3)

                    emit_scores_B(3)
                    emit_scores_A(3, 0); emit_pv_j(2, 0)
                    emit_scores_A(3, 1); emit_pv_j(2, 1)
                    emit_scores_A(3, 2); emit_pv_j(2, 2)
                    emit_scores_A(3, 3); emit_pv_j(2, 3)
                    emit_scores_win(3, 0)
                    emit_scores_win(3, 1)
                    emit_scores_win(3, 2)
                    emit_scores_win(3, 3)

                    emit_pv_j(3, 0)
                    emit_pv_j(3, 1)
                    emit_pv_j(3, 2)
                    emit_pv_j(3, 3)

            if loop_n == -1:
                # straight-line unroll for TimelineSim (no hw loop support)
                for i in range(6):
                    emit_exec(i % 2)
            elif loop_n:
                # unroll-K body (double-buffered inputs via si alternation)
                # to amortize the For_i back-edge pipeline drain
                K = 8 if loop_n % 8 == 0 else 2
                assert loop_n % K == 0, "loop_n must be a multiple of 2"
                with tc.For_i(0, loop_n // K, 1):
                    for i in range(K):
                        emit_exec(i % 2)
            else:
                emit_exec(0)
    nc.finalize()
    return nc


def _prepare_inputs(hidden_states, attention_mask, Wq, bq, Wk, bk, Wv, bv, sparse_mask):
    bf = ml_dtypes.bfloat16
    hs = np.asarray(hidden_states, np.float32)
    am = np.asarray(attention_mask, np.float32).reshape(2, L)
    Wq = np.asarray(Wq, np.float32)
    Wk = np.asarray(Wk, np.float32)
    Wv = np.asarray(Wv, np.float32)
    bq = np.asarray(bq, np.float32)
    bk = np.asarray(bk, np.float32)
    bv = np.asarray(bv, np.float32)
    gA, gB = _glob_cols()

    in_maps = []
    per_batch = {}
    fast = True
    for b in range(2):
        ht = np.ascontiguousarray(hs[b].T).astype(bf)  # [1024, 2048]
        # per-block self-window bias: -1e4 at excluded keys (A cols 120..127
        # always; k=0 when bk>=1 since col 128bk is a B global), plus the
        # additive attention mask at key 128bk+k.
        mbias = np.zeros((128, 16), np.float32)
        for blk in range(NB):
            mbias[:, blk] = am[b][128 * blk : 128 * blk + 128]
            mbias[120:128, blk] += NEG
            if blk >= 1:
                mbias[0, blk] += NEG
        if not np.all(mbias[:, 1:] == mbias[:, 1:2]):
            fast = False
        per_batch[b] = (
            ht,
            mbias,
            am[b][gA].reshape(128, 1).copy(),
            _rep_attnB(am[b][gB]),
        )

    for core in range(8):
        b, g = core // 4, core % 4
        ht, mbias, aAv, aBv = per_batch[b]
        cols = slice(256 * g, 256 * g + 256)
        wq = (Wq[:, cols] * 0.125).astype(bf)
        wk_ = Wk[:, cols].astype(bf)
        wv_ = np.zeros((HID, 260), np.float32)
        for j in range(4):
            wv_[:, 65 * j : 65 * j + 64] = Wv[:, cols.start + 64 * j : cols.start + 64 * j + 64]
        bqk_ = np.stack(
            [
                bq[cols][:128] * 0.125,
                bq[cols][128:] * 0.125,
                bk[cols][:128],
                bk[cols][128:],
            ],
            axis=1,
        ).astype(np.float32)
        in_maps.append(
            dict(
                ht=ht,
                wq=wq,
                wk=wk_,
                wv=wv_.astype(bf),
                bqk=np.ascontiguousarray(bqk_),
                attnA=aAv,
                attnB=aBv,
                mbias=np.ascontiguousarray(mbias),
            )
        )
    # NOTE: bv is folded nowhere: it is zeros by construction in this problem.
    assert np.all(bv == 0.0), "kernel assumes zero V bias"
    return in_maps, fast


def kernel(hidden_states, attention_mask, Wq, bq, Wk, bk, Wv, bv, sparse_mask,
           trace=False):
    in_maps, fast = _prepare_inputs(
        hidden_states, attention_mask, Wq, bq, Wk, bk, Wv, bv, sparse_mask
    )
    key = ("nc", fast)
    if key not in _prog_cache:
        _prog_cache[key] = build_program(fast_bias=fast)
    nc = _prog_cache[key]
    res = run_bass_kernel_spmd(nc, in_maps, list(range(8)), trace=trace)
    out = np.empty((2, L, HID), np.float32)
    for core in range(8):
        b, g = core // 4, core % 4
        out[b][:, 256 * g : 256 * g + 256] = np.asarray(
            res.results[core]["out"], np.float32
        )
    if trace:
        _prog_cache["last_results"] = res
    return out
